# revision 1
# baseline (speedup 1.0000x reference)
"""Trainium2 Bass kernel for the STFT denoiser.

Pipeline per shard (8 shards = 4 batches x 2 time-halves, one per NeuronCore):
  1. PE-transpose audio into sample-interleaved SBUF layout X[s%128, s//128].
  2. Forward STFT as fp32r matmuls: ft[c, f] (channels on partitions, 9 c-tiles).
  3. Pointwise denoise: scale = relu(1 - 0.1*bias/mag) applied to re/im.
  4. Inverse STFT fused with overlap-add: out_block[g, r] accumulated in PSUM
     over (c-chunk, shift) via shifted rec columns x extended inverse basis.
  5. Multiply by 4/window_sumsquare, DMA out.
"""
import sys
for _p in ("/opt/trn_rl_repo", "/root/.axon_site/_ro/trn_rl_repo"):
    if _p not in sys.path:
        sys.path.insert(0, _p)

import os

import numpy as np

import concourse.bass as bass
import concourse.tile as tile
import concourse.mybir as mybir
from concourse import bacc
from concourse.bass_utils import run_bass_kernel_spmd
from concourse.masks import make_identity

F32 = mybir.dt.float32
F16 = mybir.dt.float16

N_FFT = 1024
HOP = 256
CUT = 513            # n_fft//2 + 1
B = 4
T = 1048576
PAD = 512
TP = T + 2 * PAD     # padded length 1049600
F_TOTAL = 4097       # global frames

NF = 2052            # frames computed per shard (incl. phantom edges)
FT = 342             # frames per forward tile (6 tiles)
FTILES = [(342 * i, 342) for i in range(6)]
MASKW = 684          # mask columns: [0:342] tile0, [342:684] last tile
XS_LEN = HOP * (NF - 1) + N_FFT          # 526080 input samples per shard
XW = XS_LEN // 128                       # 4110 interleaved words
NBLK = (XW + 127) // 128                 # 33 transpose blocks
XWP = NBLK * 128                         # 4224 padded (xbar wants x128 cols)
NGT = 16                                 # inverse g-tiles (128 blocks of 256 samples)
DELTA = 3                                # rec column offset: col = g + DELTA - s

_cache = {}


def _make_host_constants():
    if "fwdT" in _cache:
        return
    n = np.arange(N_FFT)
    win = 0.5 - 0.5 * np.cos(2.0 * np.pi * n / N_FFT)
    fb = np.fft.fft(np.eye(N_FFT))
    FB = np.vstack([fb[:CUT].real, fb[:CUT].imag])          # [1026, 1024]
    scale = N_FFT / HOP
    fwd = (FB * win[None, :]).astype(np.float32)
    inv = (np.linalg.pinv(scale * FB).T * win[None, :]).astype(np.float32)
    win_sq = (win ** 2).astype(np.float32)

    # channel permutation: re_0..511, im_0..511, re_512, im_512
    perm = list(range(0, 512)) + list(range(513, 1025)) + [512, 1025]
    FWDP = fwd[perm]                                        # [1026, 1024]
    INVP = inv[perm]

    # forward lhsT: fwdT[i, k, c] = FWDP[c, 128k + i]
    fwdT = np.ascontiguousarray(
        FWDP.reshape(1026, 8, 128).transpose(2, 1, 0)).astype(np.float32)
    # inverse rhs: invE[i, ct, s, r]
    invE = np.zeros((128, 9, 4, 256), dtype=np.float32)
    for ct in range(8):
        seg = INVP[128 * ct:128 * ct + 128].reshape(128, 4, 256)
        invE[:, ct] = seg
    invE[0:2, 8] = INVP[1024:1026].reshape(2, 4, 256)

    # window sumsquare -> 4/ws
    n_len = N_FFT + HOP * (F_TOTAL - 1)
    ws = np.zeros(n_len, dtype=np.float64)
    for s in range(4):
        # frame f covers [256f, 256f+1024); sum win_sq at every hop offset
        pass
    idx = (np.arange(F_TOTAL)[:, None] * HOP + np.arange(N_FFT)[None, :]).ravel()
    np.add.at(ws, idx, np.tile(win_sq.astype(np.float64), F_TOTAL))
    ws = ws.astype(np.float32)
    tiny = np.finfo(np.float32).tiny
    invws_g = np.where(ws > tiny, np.float32(4.0) / ws, np.float32(4.0)).astype(np.float32)

    # per-half constants
    invws3 = {}
    mask = {}
    for j in (0, 1):
        Bj = 2048 * j + 2
        arr = np.empty((128, 3, 256), dtype=np.float32)
        g = np.arange(128)
        for col, gt in ((0, 0), (1, 1), (2, 15)):
            base = (Bj + 128 * gt + g) * 256
            arr[:, col, :] = invws_g[base[:, None] + np.arange(256)[None, :]]
        invws3[j] = arr
        m = np.ones((128, MASKW), dtype=np.float32)
        if j == 0:
            m[:, 0] = 0.0          # phantom frame m=0 (global f=-1)
        else:
            m[:, 342 + (2050 - 1710)] = 0.0  # phantom frame m=2050 (global f=4097)
            m[:, 342 + (2051 - 1710)] = 0.0  # unused frame m=2051
        mask[j] = m

    _cache.update(fwdT=fwdT.astype(np.float16), invE=invE.astype(np.float16),
                  invws3=invws3, mask=mask, win=win)


def _build_nc():
    if "nc" in _cache:
        return _cache["nc"]
    nc = bacc.Bacc("TRN2", target_bir_lowering=False, debug=False, num_devices=8)

    xs_d = nc.dram_tensor("xs", [128, XWP], F16, kind="ExternalInput")
    fwdT_d = nc.dram_tensor("fwdT", [128, 8, 1026], F16, kind="ExternalInput")
    invE_d = nc.dram_tensor("invE", [128, 9, 4, 256], F16, kind="ExternalInput")
    bias5_d = nc.dram_tensor("bias5", [128, 5], F32, kind="ExternalInput")
    mask_d = nc.dram_tensor("mask", [128, MASKW], F32, kind="ExternalInput")
    invws3_d = nc.dram_tensor("invws3", [128, 3, 256], F32, kind="ExternalInput")
    out_d = nc.dram_tensor("out", [2048, 256], F32, kind="ExternalOutput")

    CT_ORDER = [0, 4, 1, 5, 2, 6, 3, 7, 8]

    with tile.TileContext(nc) as tc:
        with (
            tc.tile_pool(name="const", bufs=1) as cpool,
            tc.tile_pool(name="big", bufs=1) as bigp,
            tc.tile_pool(name="tmp", bufs=2) as tmp,
            tc.tile_pool(name="ob", bufs=2) as obp,
            tc.tile_pool(name="psf", bufs=6, space="PSUM") as psf,
            tc.tile_pool(name="psi", bufs=2, space="PSUM") as psi,
        ):
            eps = cpool.tile([128, 1], F32)
            nc.gpsimd.memset(eps[:], 1e-30)
            fwdT = cpool.tile([128, 8, 1026], F16)
            invE = cpool.tile([128, 9, 4, 256], F16)
            bias5 = cpool.tile([128, 5], F32)
            maskt = cpool.tile([128, MASKW], F32)
            invws3 = cpool.tile([128, 3, 256], F32)
            # constants go down the Pool (SWDGE) queue so the SP queue can
            # start streaming audio blocks immediately; chunked so the first
            # forward matmuls only wait for their own k-chunk
            nc.gpsimd.dma_start(bias5[:], bias5_d.ap())
            nc.gpsimd.dma_start(maskt[:], mask_d.ap())
            for k in range(0, 8, 2):
                nc.gpsimd.dma_start(fwdT[:, k:k + 2, :], fwdT_d.ap()[:, k:k + 2, :])
            nc.gpsimd.dma_start(invws3[:], invws3_d.ap())

            X = bigp.tile([128, XWP], F16)
            REC = bigp.tile([128, 8, NF], F16)
            rec8t = bigp.tile([2, NF], F16)    # denoised rec of tile 8

            # ---- stage 1: load host-interleaved audio straight into X ----
            for c0 in range(0, XWP, 1056):
                cw = min(1056, XWP - c0)
                nc.sync.dma_start(X[:, c0:c0 + cw], xs_d.ap()[:, c0:c0 + cw])

            X3 = X[:].rearrange("p (w e) -> p w e", e=2)
            out_v = out_d.ap().rearrange("(t g) r -> g t r", g=128)

            def inverse_gtile(gt):
                q = psi.tile([128, 256], F32, tag="inv", name="q")
                first = True
                for ct in range(9):
                    for s in range(4):
                        c0 = 128 * gt + DELTA - s
                        lhsT = (REC[:, ct, c0:c0 + 128] if ct < 8
                                else rec8t[0:2, c0:c0 + 128])
                        rhs = (invE[:, ct, s, :] if ct < 8
                               else invE[0:2, 8, s, :])
                        nc.tensor.matmul(q[:, :], lhsT, rhs,
                                         start=first, stop=(ct == 8 and s == 3))
                        first = False
                osb = obp.tile([128, 256], F32, tag="osb", name="osb")
                wsel = 0 if gt == 0 else (2 if gt == NGT - 1 else 1)
                nc.vector.tensor_mul(osb[:], q[:, :], invws3[:, wsel, :])
                nc.sync.dma_start(out_v[:, gt, :], osb[:])

            # ---- stage 2+3: forward STFT + denoise, inverse interleaved ----
            gt_done = 0
            for fti, (m0, W) in enumerate(FTILES):
                if fti < 3:
                    nc.gpsimd.dma_start(invE[:, 3 * fti:3 * fti + 3],
                                        invE_d.ap()[:, 3 * fti:3 * fti + 3])
                if fti == 0:
                    msk = maskt[:, 0:342]
                elif fti == len(FTILES) - 1:
                    msk = maskt[:, 342:684]
                else:
                    msk = None
                ps = {}
                for ct in CT_ORDER:
                    p = psf.tile([128, W], F32, tag="fwd", name="p")
                    ps[ct] = p
                    lo, hi = (128 * ct, 128 * ct + 128) if ct < 8 else (1024, 1026)
                    o = p[:, :] if ct < 8 else p[0:2, :]
                    for k in range(8):
                        rhs = X3[:, m0 + k // 2: m0 + k // 2 + W, k % 2]
                        nc.tensor.matmul(o, fwdT[:, k, lo:hi], rhs,
                                         start=(k == 0), stop=(k == 7))
                # pointwise per pair (3 rotating temp tags: a, b, c)
                for pr in range(4):
                    pre, pim = ps[pr], ps[pr + 4]
                    ta = tmp.tile([128, W], F32, tag="ta", name="ta")
                    tb = tmp.tile([128, W], F32, tag="tb", name="tb")
                    tcc = tmp.tile([128, W], F32, tag="tc", name="tcc")
                    nc.scalar.square(ta[:], pre[:])
                    nc.scalar.square(tb[:], pim[:])
                    nc.vector.tensor_add(tcc[:], ta[:], tb[:])      # msq
                    nc.scalar.activation(ta[:], tcc[:],
                                         mybir.ActivationFunctionType.Sqrt,
                                         bias=eps[:, 0:1])          # mag
                    nc.vector.reciprocal_approx_fast(out=tb[:], in_=ta[:])  # rmag
                    scl = tcc
                    nc.scalar.activation(scl[:], tb[:],
                                         mybir.ActivationFunctionType.Relu,
                                         bias=1.0, scale=bias5[:, pr:pr + 1])
                    if msk is not None:
                        nc.vector.tensor_mul(scl[:], scl[:], msk)
                    nc.vector.tensor_mul(REC[:, pr, m0:m0 + W], pre[:], scl[:])
                    nc.vector.tensor_mul(REC[:, pr + 4, m0:m0 + W], pim[:], scl[:])
                # tile 8 pointwise straight from PSUM
                a8 = tmp.tile([2, W], F32, tag="ta", name="a8")
                b8 = tmp.tile([2, W], F32, tag="tb", name="b8")
                nc.scalar.activation(b8[:], ps[8][0:2, :],
                                     mybir.ActivationFunctionType.Abs,
                                     bias=eps[0:2, 0:1])            # |x|+eps
                nc.vector.reciprocal_approx_fast(out=a8[:], in_=b8[:])
                nc.scalar.activation(b8[:], a8[:],
                                     mybir.ActivationFunctionType.Relu,
                                     bias=1.0, scale=bias5[0:2, 4:5])
                if msk is not None:
                    nc.vector.tensor_mul(b8[:], b8[:], msk[0:2, :])
                nc.vector.tensor_mul(rec8t[:, m0:m0 + W], ps[8][0:2, :], b8[:])
                # inverse g-tiles whose rec columns are now complete
                avail = m0 + W
                while (fti >= 3 and gt_done < NGT
                       and 128 * gt_done + 131 <= avail):
                    inverse_gtile(gt_done)
                    gt_done += 1
            while gt_done < NGT:
                inverse_gtile(gt_done)
                gt_done += 1

    nc.compile()
    _cache["nc"] = nc
    return nc


def _prep_inputs(audio, bias_spec):
    _make_host_constants()
    bias = np.asarray(bias_spec, dtype=np.float32).reshape(CUT)
    bias5 = np.zeros((128, 5), dtype=np.float32)
    for c in range(4):
        bias5[:, c] = -0.1 * bias[128 * c:128 * c + 128]
    bias5[0:2, 4] = -0.1 * bias[512]

    in_maps = []
    for b in range(B):
        xp = np.pad(np.asarray(audio[b], dtype=np.float32), PAD, mode="reflect")
        for j in (0, 1):
            if j == 0:
                xs = np.concatenate([np.zeros(256, np.float32), xp[0:XS_LEN - 256]])
            else:
                f0 = 2047
                start = HOP * f0
                xs = np.concatenate([xp[start:], np.zeros(512, np.float32)])
            assert xs.size == XS_LEN
            xsh = np.zeros(XWP * 128, dtype=np.float16)
            xsh[:XS_LEN] = xs.astype(np.float16)
            xst = np.ascontiguousarray(xsh.reshape(XWP, 128).T)
            in_maps.append({
                "xs": xst,
                "fwdT": _cache["fwdT"],
                "invE": _cache["invE"],
                "bias5": bias5,
                "mask": _cache["mask"][j],
                "invws3": _cache["invws3"][j],
            })
    return in_maps


def kernel(audio, bias_spec, _trace=False):
    nc = _build_nc()
    in_maps = _prep_inputs(audio, bias_spec)
    res = run_bass_kernel_spmd(nc, in_maps, core_ids=list(range(8)), trace=_trace)
    out = np.empty((B, 1, T), dtype=np.float32)
    for b in range(B):
        for j in (0, 1):
            shard = res.results[2 * b + j]["out"].reshape(-1)
            out[b, 0, 524288 * j: 524288 * (j + 1)] = shard
    if _trace:
        kernel.last_results = res
    return out

